# revision 1
# baseline (speedup 1.0000x reference)
"""Low-rank RNN Bass kernel v3 — time-parallel, rebalanced windows.

Core 0 computes traj t in [0, 99) exactly from x0 (no warmup needed);
cores k>=1 compute t in [Tk, Tk+59), Tk = 99 + 59*(k-1), warming up
WARM=40 steps from x=0.  Every core runs the same 100-step program:
  ti=0   pseudo-step: x_1 = xinit (injected via noise slot 0;
         core 0: x0, others: 0)
  ti=1..99  dynamics step; drive slot ti maps to global t = t_start+ti-1
All 99 tanh slots are projected; host keeps 99 (core 0) / last 59 (k>0).

Per-step engines (see kernel2 docstring for derivation):
  ACT  r = tanh(x)        [128,512] psum->sbuf bf16
  DVE  xn = 0.8x + n'     psum->sbuf bf16 (off critical path)
  PE   z = N^T r          8 bf16 mm -> psum [4,64]
  DVE  zs = bf16(z)       -> sbuf
  PE   x' group: Win@u (8mm) + Ident@xn + M~@zs (8mm); outproj interleaved
"""

import os

import numpy as np

ABLATE = set(filter(None, os.environ.get("K3_ABLATE", "").split(",")))
OB = int(os.environ.get("K3_OB", "4"))    # outproj steps per batched call

B, T, I, H, O, R = 64, 512, 16, 1024, 8, 4
NCORES = 8
WARM = int(os.environ.get("K3_WARM", "24"))
W0 = WARM + (T - WARM) // NCORES      # 99 = core-0 window
WK = W0 - WARM                        # 59 = window for cores 1..7
NSTEP = W0                            # 99 dynamics steps
NT = NSTEP + 1                        # 100 drive slots (slot 0 = injection)
NOUT = NSTEP                          # 99 projected slots per core
HC = H // 128
CB = HC * B                           # 512
NCH = 10                              # noise steps per DMA chunk
NCHUNKS = (NT + NCH - 1) // NCH
NTP = NCHUNKS * NCH                   # padded drive slots (host zero-pads)
OGS = 8                               # outproj steps per PSUM group
NOG = (NOUT + OGS - 1) // OGS         # 13 (last partial: 3)
TAU = 0.2
NOISE_STD = 0.05

assert W0 + (NCORES - 1) * WK == T

_cache = {}


def _build(timing_reps=1):
    from contextlib import nullcontext

    import concourse.bacc as bacc
    import concourse.mybir as mybir
    import concourse.tile as tile

    FP = mybir.dt.float32
    BF = mybir.dt.bfloat16
    Tanh = mybir.ActivationFunctionType.Tanh
    mult = mybir.AluOpType.mult
    add = mybir.AluOpType.add

    nc = bacc.Bacc("TRN2", target_bir_lowering=False, debug=False)

    uT_d = nc.dram_tensor("uT", [I + 1, NT * B], BF, kind="ExternalInput")
    noise_d = nc.dram_tensor("noiseT", [128, NTP * CB], BF, kind="ExternalInput")
    n1p_d = nc.dram_tensor("N1p", [128, HC * R], BF, kind="ExternalInput")
    m2b_d = nc.dram_tensor("M2b", [R, H], BF, kind="ExternalInput")
    winT_d = nc.dram_tensor("WinT", [I + 1, H], BF, kind="ExternalInput")
    id_d = nc.dram_tensor("IdentB", [128, 128], BF, kind="ExternalInput")
    woutT_d = nc.dram_tensor("WoutT", [128, HC * O], BF, kind="ExternalInput")
    woutb_d = nc.dram_tensor("Woutb", [O, 1], FP, kind="ExternalInput")
    out_d = nc.dram_tensor("outT", [O, NOUT * B], FP, kind="ExternalOutput")

    with tile.TileContext(nc) as tc:
        with (
            tc.tile_pool(name="const", bufs=1) as constp,
            tc.tile_pool(name="noisep", bufs=3) as noisep,
            tc.tile_pool(name="rbufp", bufs=1) as rbufp,
            tc.tile_pool(name="xnp", bufs=2) as xnp,
            tc.tile_pool(name="zsp", bufs=2) as zsp,
            tc.tile_pool(name="outp", bufs=2) as outp,
            tc.tile_pool(name="psx", bufs=2, space="PSUM") as psx,
            tc.tile_pool(name="psz", bufs=2, space="PSUM") as psz,
            tc.tile_pool(name="pso", bufs=2, space="PSUM") as pso,
        ):
            uT = constp.tile([I + 1, NT * B], BF)
            nc.sync.dma_start(uT[:], uT_d[:])
            N1p = constp.tile([128, HC * R], BF)
            nc.sync.dma_start(N1p[:], n1p_d[:])
            M2b = constp.tile([R, H], BF)
            nc.sync.dma_start(M2b[:], m2b_d[:])
            WinT = constp.tile([I + 1, H], BF)
            nc.sync.dma_start(WinT[:], winT_d[:])
            IdentB = constp.tile([128, 128], BF)
            nc.sync.dma_start(IdentB[:], id_d[:])
            WoutT = constp.tile([128, HC * O], BF)
            nc.sync.dma_start(WoutT[:], woutT_d[:])
            Woutb = constp.tile([O, 1], FP)
            nc.sync.dma_start(Woutb[:], woutb_d[:])

            RB = 8
            rbuf = rbufp.tile([128, RB * CB], BF)

            loop_cm = (
                tc.For_i(0, timing_reps, 1) if timing_reps > 1 else nullcontext()
            )
            loop_cm.__enter__()

            nchunks = {}

            def prefetch(ci):
                if ci < NCHUNKS and ci not in nchunks:
                    nwt = noisep.tile([128, NCH * CB], BF, tag="nw")
                    nc.sync.dma_start(
                        nwt[:], noise_d[:, ci * NCH * CB : (ci + 1) * NCH * CB]
                    )
                    nchunks[ci] = nwt

            def nslice(ti):
                ci = ti // NCH
                return nchunks[ci][:, (ti % NCH) * CB : (ti % NCH + 1) * CB]

            def rslot(s):
                return rbuf[:, (s % RB) * CB : (s % RB + 1) * CB]

            prefetch(0)
            prefetch(1)

            po_tiles = {}

            r4 = rbuf[:].rearrange("p (s c b) -> p s c b", c=HC, b=B)

            def outproj(s, nb=1):
                """Project r_s..r_{s+nb-1}; slot s in [2, NSTEP+1]."""
                idx = s - 2
                g, pos = idx // OGS, idx % OGS
                assert pos + nb <= OGS
                glen = min(OGS, NOUT - g * OGS)
                if pos == 0:
                    po_tiles[g] = pso.tile([O, OGS * B], FP, tag="po", name="po")
                po = po_tiles[g]
                s0 = s % RB
                for c in range(HC):
                    nc.tensor.matmul(
                        po[:, pos * B : (pos + nb) * B],
                        WoutT[:, c * O : (c + 1) * O],
                        r4[:, s0 : s0 + nb, c, :],
                        start=(c == 0 and pos == 0),
                        stop=(c == HC - 1 and pos + nb == glen),
                    )
                if pos + nb == glen:
                    ob = outp.tile([O, OGS * B], FP, tag="ob")
                    nc.vector.tensor_scalar_add(
                        ob[:, : glen * B], po[:, : glen * B], Woutb[:, 0:1]
                    )
                    nc.sync.dma_start(
                        out_d[:, g * OGS * B : (g * OGS + glen) * B],
                        ob[:, : glen * B],
                    )

            # ti=0 pseudo-step: x_1 = xinit (noise slot 0)
            xg = psx.tile([128, CB], FP, tag="xg")
            nc.tensor.matmul(xg[:], IdentB[:], nslice(0), start=True, stop=True)
            x_prev = xg
            op_next = 2  # next unprojected r slot

            for ti in range(1, NSTEP + 1):
                if ti % NCH == 1:
                    prefetch(ti // NCH + 2)

                # batched outproj (lagged): project ring-aligned batches
                if "noout" not in ABLATE:
                    while op_next < ti and (
                        (op_next % OB != 0 and op_next + 1 <= ti)
                        or (op_next % OB == 0 and op_next + OB <= ti)
                    ):
                        nb = OB if op_next % OB == 0 else 1
                        # never let a batch cross an OGS group boundary
                        pos = (op_next - 2) % OGS
                        nb = min(nb, OGS - pos, NSTEP + 2 - op_next)
                        outproj(op_next, nb)
                        op_next += nb

                xg = psx.tile([128, CB], FP, tag="xg")

                # drive: Win @ u_ti
                ut = uT[:, ti * B : (ti + 1) * B]
                for c in range(HC if "nowin" not in ABLATE else 1):
                    nc.tensor.matmul(
                        xg[:, c * B : (c + 1) * B],
                        WinT[:, c * 128 : (c + 1) * 128],
                        ut,
                        start=(c == 0),
                        stop=False,
                    )

                # xn = 0.8 * x_prev + n' (DVE: GPSIMD cannot touch PSUM)
                xn = xnp.tile([128, CB], BF, tag="xn")
                nt = nslice(ti)
                if "noxn" in ABLATE:
                    nc.vector.tensor_copy(xn[:], nt[:])
                else:
                    nc.vector.scalar_tensor_tensor(
                        xn[:], x_prev[:], 1.0 - TAU, nt[:], op0=mult, op1=add
                    )

                # r_ti = tanh(x_ti)
                rs = rslot(ti)
                if "notanh" in ABLATE:
                    rs = rslot(0)
                elif "tsplit" in ABLATE:
                    nc.scalar.activation(
                        rs[:, : CB // 2], x_prev[:, : CB // 2], Tanh
                    )
                    nc.scalar.activation(
                        rs[:, CB // 2 :], x_prev[:, CB // 2 :], Tanh
                    )
                else:
                    nc.scalar.activation(rs, x_prev[:], Tanh)

                if "norec" in ABLATE:
                    nc.tensor.matmul(
                        xg[:], IdentB[:], xn[:], start=False, stop=True
                    )
                    x_prev = xg
                    continue

                # z = sum_c N_c^T r_c
                z = psz.tile([R, B], FP, tag="z")
                for c in range(HC):
                    nc.tensor.matmul(
                        z[:],
                        N1p[:, c * R : (c + 1) * R],
                        rs[:, c * B : (c + 1) * B],
                        start=(c == 0),
                        stop=(c == HC - 1),
                    )

                # ident @ xn between mm1 and mm2 (fills the zs wait)
                nc.tensor.matmul(xg[:], IdentB[:], xn[:], start=False, stop=False)

                # zs = bf16(z)
                zs = zsp.tile([R, B], BF, tag="zs")
                nc.vector.tensor_copy(zs[:], z[:])

                # x_{ti+1} += M~ @ zs
                for c in range(HC):
                    nc.tensor.matmul(
                        xg[:, c * B : (c + 1) * B],
                        M2b[:, c * 128 : (c + 1) * 128],
                        zs[:],
                        start=False,
                        stop=(c == HC - 1),
                    )
                x_prev = xg

            # final r = tanh(x_{NSTEP+1}), remaining outprojs
            nc.scalar.activation(rslot(NSTEP + 1), x_prev[:], Tanh)
            while op_next <= NSTEP + 1:
                pos = (op_next - 2) % OGS
                nb = 1 if op_next % OB else min(
                    OB, OGS - pos, NSTEP + 2 - op_next
                )
                nb = max(1, min(nb, NSTEP + 2 - op_next))
                outproj(op_next, nb)
                op_next += nb
            loop_cm.__exit__(None, None, None)

    nc.compile()
    return nc


def _get_nc():
    if "nc" not in _cache:
        _cache["nc"] = _build()
    return _cache["nc"]


def _t_start(k):
    return 0 if k == 0 else W0 + WK * (k - 1) - WARM


def _host_prep(u, x0, noise, M, N, Win_w, Win_b, Wout_w, Wout_b):
    import ml_dtypes

    bf = ml_dtypes.bfloat16
    f = np.float32

    n_chunks = N.reshape(HC, 128, R).transpose(1, 0, 2)
    N1p = np.ascontiguousarray(n_chunks.reshape(128, HC * R)).astype(bf)
    M2b = np.ascontiguousarray((TAU / H) * M.T).astype(bf)
    WinT = np.concatenate(
        [TAU * Win_w.T.astype(f), (TAU * Win_b).astype(f)[None, :]], axis=0
    ).astype(bf)
    IdentB = np.eye(128, dtype=f).astype(bf)
    WoutT = np.ascontiguousarray(
        Wout_w.T.reshape(HC, 128, O).transpose(1, 0, 2).reshape(128, HC * O)
    ).astype(bf)
    Woutb = np.ascontiguousarray(Wout_b.astype(f)[:, None])

    x0T = np.ascontiguousarray(
        x0.T.reshape(HC, 128, B).transpose(1, 0, 2).reshape(128, CB), dtype=f
    )

    in_maps = []
    for k in range(NCORES):
        ts = _t_start(k)
        # drive slots ti=1..NSTEP -> global t = ts+ti-1 in [ts, ts+NSTEP)
        uw = np.zeros((I + 1, NT, B), dtype=f)
        uw[:I, 1:] = u[:, ts : ts + NSTEP].transpose(2, 1, 0)
        uw[I, 1:] = 1.0
        uT = uw.reshape(I + 1, NT * B).astype(bf)

        nw = np.zeros((NTP, B, H), dtype=f)
        nw[1:NT] = NOISE_STD * noise[ts : ts + NSTEP]
        nT = np.ascontiguousarray(
            nw.reshape(NTP, B, HC, 128).transpose(3, 0, 2, 1).reshape(128, NTP * CB)
        )
        if k == 0:
            nT[:, 0:CB] = x0T  # exact x0 injection
        in_maps.append(
            {
                "uT": uT,
                "noiseT": nT.astype(bf),
                "N1p": N1p,
                "M2b": M2b,
                "WinT": WinT,
                "IdentB": IdentB,
                "WoutT": WoutT,
                "Woutb": Woutb,
            }
        )
    return in_maps


def _assemble(core_outs):
    """core_outs[k]: [O, NOUT*B] -> full (B, T, O)."""
    out = np.empty((B, T, O), dtype=np.float32)
    for k, outT in enumerate(core_outs):
        tr = outT.reshape(O, NOUT, B).transpose(2, 1, 0)  # (B, NOUT, O)
        if k == 0:
            out[:, 0:W0] = tr
        else:
            t0 = W0 + WK * (k - 1)
            out[:, t0 : t0 + WK] = tr[:, WARM:]
    return out


last_results = None


def kernel(u, x0, noise, M, N, Win_w, Win_b, Wout_w, Wout_b):
    from concourse.bass_utils import run_bass_kernel_spmd

    global last_results
    nc = _get_nc()
    in_maps = _host_prep(u, x0, noise, M, N, Win_w, Win_b, Wout_w, Wout_b)
    res = run_bass_kernel_spmd(nc, in_maps, core_ids=list(range(NCORES)))
    last_results = res
    return _assemble([res.results[k]["outT"] for k in range(NCORES)])


def gather_sim_outputs(sims, nc):
    out = np.full((B, T, O), np.nan, dtype=np.float32)
    for cid, sim in sims:
        outT = np.asarray(sim.mem_tensor("outT")).reshape(O, NOUT * B)
        tr = outT.reshape(O, NOUT, B).transpose(2, 1, 0)
        if cid == 0:
            out[:, 0:W0] = tr
        else:
            t0 = W0 + WK * (cid - 1)
            out[:, t0 : t0 + WK] = tr[:, WARM:]
    return out


SIM_CORES = [0, 3]

